# revision 27
# baseline (speedup 1.0000x reference)
"""AVWGCN Trainium2 kernel: adaptive-adjacency Chebyshev GCN.

Math (per core, batch-sharded over B: 8 batches/core):
  A = relu(E @ E^T) (symmetric), M = exp(A), r = rowsum(M), S = diag(1/r) M
  X2[m,(b,c)] = x[b,m,c]
  x1 = diag(1/r) (M @ X2)            (T1 term)
  x2o = diag(1/r) (M @ x1)           (= S^2 x; T2 = 2 S^2 - I folded on host)
  out[b,n,o] = sum_d E[n,d] * ( sum_{k,i} xg_k[n,(b,i)] Wp'[d,k,i,o] + bp[d,o] )
    with Wp'_0 = Wp_0 - Wp_2, Wp'_2 = 2 Wp_2 (host fold of the Chebyshev -x
    term), so xg = [x, S x, S^2 x].
Key structure:
  - M symmetric -> its [n-part, m-free] tiles serve directly as matmul lhsT.
  - x1/x2o written interleaved into a padded tile xp[j][n, (b,128)] (x1 at
    cols b*128+c, x2o at b*128+64+c); ONE hw DMA transpose per (half, j)
    lands [x1^T; x2^T] stacked on 128 partitions = the K=128 lhsT (k1;k2).
    (XBAR semantics: each 128-col source block transposes to partitions
    0..127 = col-in-block, into successive free-ranges of the out AP.)
  - gconv via Z-form: Z[n,(o,d)] = [x1;x2]^T-pass + [x;ones]-pass against
    host-reordered weights; bias rides the ones row; epilogue: ACT casts
    Z->bf16, DVE multiplies by broadcast E and tree-reduces d, batched 4
    batches at a time.
  - batch dim processed in two halves so the epilogue (ACT/DVE-heavy)
    overlaps the second half's S-matmul phase (PE-heavy).
"""

from contextlib import ExitStack

import numpy as np

import concourse.bass as bass
import concourse.mybir as mybir
import concourse.tile as tile
from concourse.bass_utils import run_bass_kernel_spmd

B, N, C, CHEB_K, EMBED = 64, 2048, 64, 3, 16
NCORES = 8
BC = B // NCORES            # batches per core
F = BC * C                  # 512: free width of X2 [m, (b,c)]
FH = F // 2                 # 256: per-batch-half width
NT = N // 128               # 16 n-chunks
FP32 = mybir.dt.float32
BF16 = mybir.dt.bfloat16
MM_DT = BF16
DO = C * EMBED              # 1024, Z free width, (o, d) ordered


_WAIT_CAP = {"InstDMACopy": 1}
_WAIT_SAFE = {"InstEventSemaphore", "InstCall",
              "InstUnconditionalBranch", "InstISA", "InstRegisterMove"}


def _split_excess_waits(nc):
    """Walrus rejects compute instructions carrying more sync waits than the
    ISA struct can encode. Hoist excess waits onto an inserted same-engine
    Drain immediately before the instruction (semantically identical)."""
    SyncInfo = None
    n_fix = 0
    for f in nc.m.functions:
        for blk in f.blocks:
            out_insts = []
            for inst in blk.instructions:
                tn = type(inst).__name__
                si = inst.sync_info
                w = list(si.on_wait) if (si is not None and si.on_wait) else []
                cap = _WAIT_CAP.get(tn, 1)
                if tn not in _WAIT_SAFE and len(w) > cap:
                    if SyncInfo is None:
                        SyncInfo = type(si)
                    for wx in w:
                        d = mybir.InstDrain(name=f"I-wsplit{nc.next_id()}",
                                            ins=[], outs=[])
                        d.engine = inst.engine
                        d.sync_info = SyncInfo(on_wait=[wx], on_update=[])
                        out_insts.append(d)
                    si.on_wait = []
                    n_fix += 1
                out_insts.append(inst)
            blk.instructions[:] = out_insts
    return n_fix


def build_nc():
    nc = bass.Bass()
    x2d = nc.dram_tensor("x2d", [N, F], BF16, kind="ExternalInput").ap()
    xt = nc.dram_tensor("xt", [C, BC * N], BF16, kind="ExternalInput").ap()
    et = nc.dram_tensor("et", [3 * EMBED, N], BF16, kind="ExternalInput").ap()
    etlo_d = nc.dram_tensor("etlo", [3 * EMBED, N], BF16, kind="ExternalInput").ap()
    en = nc.dram_tensor("en", [N, EMBED], BF16, kind="ExternalInput").ap()
    wpfa = nc.dram_tensor("wpfa", [128, DO], BF16, kind="ExternalInput").ap()
    wpfb = nc.dram_tensor("wpfb", [65, DO], BF16, kind="ExternalInput").ap()
    out = nc.dram_tensor("out", [BC, N, C], BF16, kind="ExternalOutput").ap()

    with tile.TileContext(nc) as tc:
        with ExitStack() as ctx:
            kernel_body(ctx, tc, out, x2d, xt, et, etlo_d, en, wpfa, wpfb)
    _split_excess_waits(nc)
    return nc


def kernel_body(ctx, tc, out, x2d, xt, et, etlo_d, en, wpfa, wpfb):
    nc = tc.nc

    singles = ctx.enter_context(tc.tile_pool(name="singles", bufs=1))
    zsb_pool = ctx.enter_context(tc.tile_pool(name="zsb", bufs=3))
    outs_pool = ctx.enter_context(tc.tile_pool(name="outs", bufs=2))

    # ---- constants / inputs ----
    # split-precision E^T, K-stacked: one K=48 matmul computes
    # EhiEhi^T + EhiElo^T + EloEhi^T (lhsT=[Ehi;Ehi;Elo], rhs=[Ehi;Elo;Ehi])
    et_ctx = ExitStack()
    et_pool = et_ctx.enter_context(tc.tile_pool(name="etp", bufs=1))
    ethi = et_pool.tile([3 * EMBED, N], MM_DT, tag="ethi")
    etlo = et_pool.tile([3 * EMBED, N], MM_DT, tag="etlo")
    # split first chunks so the first stage-1 matmul starts ASAP
    nc.gpsimd.dma_start(out=ethi[:, 0:128], in_=et[:, 0:128])
    nc.gpsimd.dma_start(out=etlo[:, 0:1024], in_=etlo_d[:, 0:1024])
    nc.gpsimd.dma_start(out=ethi[:, 128:N], in_=et[:, 128:N])
    nc.gpsimd.dma_start(out=etlo[:, 1024:N], in_=etlo_d[:, 1024:N])
    wa_sb = singles.tile([128, DO], MM_DT, tag="wa")   # rows: [k1; 2*k2]
    nc.gpsimd.dma_start(out=wa_sb, in_=wpfa)
    wb_sb = singles.tile([65, DO], MM_DT, tag="wb")    # rows: [k0-k2; bias]
    nc.gpsimd.dma_start(out=wb_sb, in_=wpfb)

    # xgt0: k0 rows (x^T from host) + ones row for the bias
    xgt0 = singles.tile([65, BC * N], MM_DT, tag="xgt0")
    nc.gpsimd.dma_start(out=xgt0[0:C, :], in_=xt)
    nc.vector.memset(xgt0[64:65, :], 1.0)
    # E chunks for the epilogue: en_sb[p, j, d] = E[j*128+p, d]
    en_sb = singles.tile([128, NT, EMBED], MM_DT, tag="en_sb")
    nc.gpsimd.dma_start(out=en_sb, in_=en.rearrange("(j p) d -> p j d", j=NT))
    # x12t: [x1^T; x2^T] stacked on partitions, filled by DMA transposes
    x12t = singles.tile([128, BC * N], MM_DT, tag="x12t")
    x12t_v = x12t.rearrange("p (b n) -> p b n", b=BC)

    # ---- stage 2 (issued early so DMA queues/sems drain during stage 1) ----
    x2_sb = [singles.tile([128, F], MM_DT, name=f"x2_{a}", tag=f"x2{a}")
             for a in range(NT)]
    for a in range(NT):
        nc.gpsimd.dma_start(out=x2_sb[a], in_=x2d[a * 128:(a + 1) * 128, :])

    # ---- stage 1: M = exp(relu(E E^T)) as bf16 tiles + row sums ----
    # M is symmetric: row-chunks 0..7 compute all columns (m_top); chunks
    # 8..15 compute only cols 1024:2048 (m_bot); the lower-left tile-grid
    # quarter (m_mir) is mirrored from m_top via one wide XBAR DMA transpose
    # per top row. Per-row tiles keep the dependency tracking exact.
    # A scoped full-width (4-bank) PSUM pool makes each row one wide exp.
    rsum2 = singles.tile([128, NT, 2], FP32, tag="rsum2")
    nc.vector.memset(rsum2[:, 0:8, 1], 0.0)
    m_top = [singles.tile([128, N], MM_DT, name=f"mt{c}", tag=f"mt{c}")
             for c in range(8)]
    m_bot = [singles.tile([128, 1024], MM_DT, name=f"mb{r}", tag=f"mb{r}")
             for r in range(8)]
    m_mir = singles.tile([128, 8, 1024], MM_DT, tag="m_mir")

    def m_ap(a, j):
        """lhsT tile for contraction chunk a, output chunk j: M[a-rows, j-cols]."""
        if a < 8:
            return m_top[a][:, j * 128:(j + 1) * 128]
        if j >= 8:
            return m_bot[a - 8][:, (j - 8) * 128:(j - 7) * 128]
        return m_mir[:, a - 8, j * 128:(j + 1) * 128]

    ps_s1_ctx = ExitStack()
    ps_s1 = ps_s1_ctx.enter_context(
        tc.tile_pool(name="ps_s1", bufs=2, space="PSUM"))
    for j in range(NT):
        jsl = slice(j * 128, (j + 1) * 128)
        pa = ps_s1.tile([128, N], FP32, name="pa", tag="pa")
        nq = 4 if j < 8 else 2
        for qq in range(nq):
            q0 = (0 if j < 8 else 1024) + qq * 512
            nc.tensor.matmul(pa[:, q0:q0 + 512],
                             lhsT=ethi[:, jsl], rhs=etlo[:, q0:q0 + 512],
                             start=True, stop=True)
        dst = m_top[j] if j < 8 else m_bot[j - 8]
        src = pa if j < 8 else pa[:, 1024:2048]
        # exp(relu(a)) == max(exp(a), 1); row-sum accumulated in the max op
        nc.scalar.activation(out=dst, in_=src,
                             func=mybir.ActivationFunctionType.Exp)
        nc.vector.tensor_scalar(out=dst, in0=dst, scalar1=1.0, scalar2=None,
                                op0=mybir.AluOpType.max,
                                op1=mybir.AluOpType.add,
                                accum_out=rsum2[:, j, 0:1])
        if j < 8:
            # mirror this row's right half into m_mir rows 8..15, col chunk j
            mir = bass.AP(tensor=m_mir.tensor,
                          offset=m_mir.offset + j * 128,
                          ap=[m_mir.ap[0], [1024, 8], [1, 128]])
            nc.sync.dma_start_transpose(out=mir, in_=m_top[j][:, 1024:2048])
    # row sums of the mirrored left halves of rows 8..15 (bypass-add trick
    # keeps the 4x DVE mode while only the accumulator output matters)
    for r in range(8):
        nc.vector.tensor_scalar(out=m_mir[:, r, :], in0=m_mir[:, r, :],
                                scalar1=0.0, scalar2=None,
                                op0=mybir.AluOpType.add,
                                op1=mybir.AluOpType.add,
                                accum_out=rsum2[:, 8 + r, 1:2])
    ps_s1_ctx.close()
    ps_mm = ctx.enter_context(tc.tile_pool(name="ps_mm", bufs=2, space="PSUM"))
    ps_z = ctx.enter_context(tc.tile_pool(name="ps_z", bufs=3, space="PSUM"))
    rsum = singles.tile([128, NT], FP32, tag="rsum")
    rinv = singles.tile([128, NT], FP32, tag="rinv")
    nc.vector.reduce_sum(out=rsum, in_=rsum2, axis=mybir.AxisListType.X)
    nc.vector.reciprocal(out=rinv, in_=rsum)
    # ACT-written copy so ACT consumers don't need a DVE wait
    rinv_act = singles.tile([128, NT], FP32, tag="rinv_act")
    nc.scalar.copy(out=rinv_act, in_=rinv)
    et_ctx.close()

    # xp[j]: padded per-half [n, (b4, 128)]: x1 at cols b*128+c, x2o at +64
    xp = [singles.tile([128, 4, 128], MM_DT, name=f"xp{j}", tag=f"xp{j}")
          for j in range(NT)]

    def s3_j(h, j):
        """x1[:, h-half] = diag(1/r) M X2[:, h-half] -> xp[j] cols b*128+c."""
        hsl = slice(h * FH, (h + 1) * FH)
        jsl = slice(j * 128, (j + 1) * 128)
        pm = ps_mm.tile([128, FH], FP32, tag="pm")
        for a in range(NT):
            nc.tensor.matmul(pm, lhsT=m_ap(a, j), rhs=x2_sb[a][:, hsl],
                             start=(a == 0), stop=(a == NT - 1))
        nc.scalar.activation(out=xp[j][:, :, 0:C], in_=pm,
                             func=mybir.ActivationFunctionType.Copy,
                             scale=rinv_act[:, j:j + 1])

    def s4_j(h, j):
        """x2o = diag(1/r) M x1 -> xp[j] cols b*128+64+c, then transpose."""
        jsl = slice(j * 128, (j + 1) * 128)
        pm = ps_mm.tile([128, FH], FP32, tag="pm")
        for a in range(NT):
            nc.tensor.matmul(pm, lhsT=m_ap(a, j), rhs=xp[a][:, :, 0:C],
                             start=(a == 0), stop=(a == NT - 1))
        nc.scalar.activation(out=xp[j][:, :, C:128], in_=pm,
                             func=mybir.ActivationFunctionType.Copy,
                             scale=rinv_act[:, j:j + 1])
        # one XBAR transpose: 4 blocks of 128 cols -> [x1^T; x2^T] on 128
        # partitions, into the 4 b-column ranges of x12t for this half
        nc.sync.dma_start_transpose(
            out=x12t_v[:, h * 4:(h + 1) * 4, jsl],
            in_=xp[j].rearrange("p b q -> p (b q)"))

    def s6_j(h, j, tail=False):
        """Z matmuls + epilogue for the 4 batches of half h, chunk j."""
        jsl = slice(j * 128, (j + 1) * 128)
        # E[jsl] broadcast over (4 batches, C outputs) via zero-stride dims
        erep_bc = bass.AP(tensor=en_sb.tensor, offset=en_sb.offset + j * EMBED,
                          ap=[en_sb.ap[0], [0, 4], [0, C], [1, EMBED]])

        zsb = zsb_pool.tile([128, 4, C, EMBED], MM_DT, tag="zsb")
        outt = outs_pool.tile([128, 4, C], MM_DT, tag="outt")
        for bq in range(4):
            b = h * 4 + bq
            col = b * N + j * 128
            pz = ps_z.tile([128, DO], FP32, tag="pz")
            for hh in range(2):
                sl = slice(hh * 512, (hh + 1) * 512)
                nc.tensor.matmul(pz[:, sl], lhsT=x12t[:, col:col + 128],
                                 rhs=wa_sb[:, sl], start=True, stop=False)
                nc.tensor.matmul(pz[:, sl], lhsT=xgt0[:, col:col + 128],
                                 rhs=wb_sb[:, sl], start=False, stop=True)
            nc.scalar.copy(
                out=zsb[:, bq].rearrange("p o d -> p (o d)"), in_=pz)
            if tail:
                _epilogue(zsb, outt, erep_bc, h, j, jsl, bq)
        if not tail:
            _epilogue(zsb, outt, erep_bc, h, j, jsl, None)

    def _epilogue(zsb, outt, erep_bc, h, j, jsl, bq):
        """DVE mul + d-tree-reduce + out DMA; bq=None batches all 4."""
        if bq is None:
            zv, ov = zsb, outt
            ebc = erep_bc
            osl = slice(h * 4, (h + 1) * 4)
            eng = nc.sync
        else:
            zv, ov = zsb[:, bq:bq + 1], outt[:, bq:bq + 1]
            ebc = bass.AP(tensor=erep_bc.tensor, offset=erep_bc.offset,
                          ap=[erep_bc.ap[0], [0, 1], [0, C], [1, EMBED]])
            osl = slice(h * 4 + bq, h * 4 + bq + 1)
            eng = nc.sync if bq % 2 == 0 else nc.scalar
        nc.vector.tensor_mul(zv, zv, ebc)           # in-place *E (bf16 2x)
        # tree-reduce over d (innermost, 16): 3 halvings + final add
        for hw_ in (8, 4, 2):
            nc.vector.tensor_add(zv[:, :, :, 0:hw_], zv[:, :, :, 0:hw_],
                                 zv[:, :, :, hw_:2 * hw_])
        nc.vector.tensor_add(ov, zv[:, :, :, 0], zv[:, :, :, 1])
        eng.dma_start(
            out=out[osl, jsl, :].rearrange("b n c -> n b c"), in_=ov)

    # ---- halves: s3 phase, then s4+s6 interleaved (lag 2). The last DEFER0
    # s6 chunks of half 0 drain inside half 1's s3 phase (whose PE-bound
    # window otherwise leaves ACT/DVE idle); half 1 defers only 2 (to cover
    # transpose DMA latency), with a per-batch tail epilogue to shorten the
    # final drain.
    DEFER0 = 6
    deferred = []
    for h in range(2):
        for j in range(NT):
            s3_j(h, j)
            if deferred and j >= 1 and (j - 1) % 2 == 0:
                s6_j(*deferred.pop(0))  # drain deferred s6s of previous half
        s4_j(h, 0)
        s4_j(h, 1)
        ndef = DEFER0 if h == 0 else 2
        for j in range(NT):
            if j + 2 < NT:
                s4_j(h, j + 2)
            if j >= NT - ndef:
                deferred.append((h, j))
            else:
                s6_j(h, j)
    for d in deferred:
        s6_j(*d, tail=True)

    global _DBG_TILES
    _DBG_TILES = {"x12t": x12t, "xgt0": xgt0, "m0": m_top[0], "rinv": rinv}


_DBG_TILES = None


_NC_CACHE = None


def kernel(x, node_embedding, weights_pool, bias_pool):
    global _NC_CACHE
    if _NC_CACHE is None:
        _NC_CACHE = build_nc()
    nc = _NC_CACHE

    import ml_dtypes
    bf16 = ml_dtypes.bfloat16

    x = np.asarray(x, dtype=np.float32)
    E = np.asarray(node_embedding, dtype=np.float32)
    Wp = np.asarray(weights_pool, dtype=np.float32)
    bp = np.asarray(bias_pool, dtype=np.float32)

    etf = np.ascontiguousarray(E.T)
    eth = etf.astype(bf16).astype(np.float32)
    elo = (etf - eth).astype(np.float32)
    et = np.ascontiguousarray(np.concatenate([eth, eth, elo], axis=0)).astype(bf16)
    etlo = np.ascontiguousarray(np.concatenate([eth, elo, eth], axis=0)).astype(bf16)
    # Chebyshev host fold: T2 = 2 S^2 - I  =>  k0' = W0 - W2, k2' = 2 W2
    Wp = Wp.copy()
    Wp[:, 0] -= Wp[:, 2]
    Wp[:, 2] *= 2.0
    # wpf[(k,i), (o,d)] = Wp[d,k,i,o]; pass A = [k1; k2], pass B = [k0; bias]
    wpf = np.ascontiguousarray(Wp.transpose(1, 2, 3, 0).reshape(CHEB_K * C, DO))
    wpfa = np.ascontiguousarray(wpf[64:192]).astype(bf16)
    bprow = np.ascontiguousarray(bp.T.reshape(1, DO))
    wpfb = np.ascontiguousarray(
        np.concatenate([wpf[0:64], bprow], axis=0)).astype(bf16)
    en_b = np.ascontiguousarray(E).astype(bf16)

    in_maps = []
    for c in range(NCORES):
        xc = x[BC * c:BC * (c + 1)]
        in_maps.append({
            "x2d": np.ascontiguousarray(
                xc.transpose(1, 0, 2).reshape(N, F)).astype(bf16),
            "xt": np.ascontiguousarray(
                xc.transpose(2, 0, 1).reshape(C, BC * N)).astype(bf16),
            "et": et, "etlo": etlo, "en": en_b, "wpfa": wpfa, "wpfb": wpfb,
        })
    res = run_bass_kernel_spmd(nc, in_maps, list(range(NCORES)))
    return np.concatenate(
        [res.results[c]["out"].astype(np.float32) for c in range(NCORES)], axis=0)


if __name__ == "__main__":
    rng = np.random.default_rng(0)
    inputs = {
        "x": rng.standard_normal((B, N, C), dtype=np.float32),
        "node_embedding": rng.standard_normal((N, EMBED), dtype=np.float32),
        "weights_pool": (rng.standard_normal((EMBED, CHEB_K, C, C), dtype=np.float32) * 0.1),
        "bias_pool": (rng.standard_normal((EMBED, C), dtype=np.float32) * 0.1),
    }
    got = kernel(**inputs)
    print("out", got.shape, got.dtype, np.abs(got).max())
